# revision 34
# baseline (speedup 1.0000x reference)
"""Sobel gradient magnitude kernel for Trainium2 (8 NeuronCores, batch-sharded).

out = sqrt(gx^2 + gy^2), gx/gy = 3x3 depthwise convs (zero-padded) of
x [16, 64, 256, 256] fp32.

Per-core layout (2 batches x 64 ch = 128 images of 256x256), DMA-roofline
oriented (~189us of mandatory HBM traffic per core at the modeled 360GB/s):
  - image rows on partitions, two 128-row halves side by side in the free dim
  - vertical 3-taps as banded-matrix matmuls on TensorE (fp16 in, fp32 psum);
    horizontal taps via input-shifted windows so every tap accumulates into
    the same 512-wide PSUM span (both halves in ONE bank -> 5 matmuls/image)
  - engine split sized against the 1456ns/image DMA budget:
      Pool   : fp32->fp16 input convert + guard memsets
      DVE    : gx PSUM evac (copy->f16) + gx^2 + m = gx^2+gy^2
      ACT    : gy^2 (Square, PSUM->f16) + sqrt(m) + weights DMA issue
      SP     : all bulk input/output DMAs
  - rows 127/128 of each image (cross-half seam) recomputed in a late pass
    whose steps are spread across the main loop, scattered over the output
"""

import numpy as np
from contextlib import ExitStack

import concourse.bacc as bacc
import concourse.mybir as mybir
from concourse.bass_utils import run_bass_kernel_spmd
from concourse.tile import TileContext, add_dep_helper

F32 = mybir.dt.float32
F16 = mybir.dt.float16
AF = mybir.ActivationFunctionType
OP = mybir.AluOpType

N_CORES = 8
B, C, H, W = 16, 64, 256, 256
B_LOC = B // N_CORES          # 2 batches per core
N_IMG = B_LOC * C             # 128 images per core
HALF = H // 2                 # 128 rows per half
WG = W + 2                    # guarded width (258)
PAIRS = N_IMG // 2            # 64 image pairs per core
FLUSH_DELAY = 12              # pairs between sqrt and its output DMA issue


def _tap_matrices(kern):
    """kern: [3,3]. For each horizontal tap t in {-1,0,+1} build the banded
    vertical matrix V_t[k, m] = kern[di, t+1] for k = m + di - 1 (clipped).
    Returns list of (t, V) for taps whose column is nonzero."""
    out = []
    for t in (-1, 0, 1):
        col = kern[:, t + 1]
        if not np.any(col):
            continue
        V = np.zeros((HALF, HALF), dtype=np.float32)
        for di in range(3):
            w = float(col[di])
            if w == 0.0:
                continue
            for m in range(HALF):
                k = m + di - 1
                if 0 <= k < HALF:
                    V[k, m] = w
        out.append((t, V))
    return out


def _plan(kx, ky):
    """Unique weight matrices + per-image matmul descriptors.

    Returns (mats, descs): mats = list of unique [128,128] fp32 matrices;
    descs = ordered (slot, bank, tap) with start/stop flags; bank 0 = gy,
    bank 1 = gx. Within a bank, taps sharing a slot are adjacent."""
    mats, keys = [], {}

    def slot_of(V):
        k = V.tobytes()
        if k not in keys:
            keys[k] = len(mats)
            mats.append(V)
        return keys[k]

    descs = []
    for bank, kern in ((0, ky), (1, kx)):   # bank 0 = gy, bank 1 = gx
        taps = [(slot_of(V), t) for t, V in _tap_matrices(kern)]
        taps.sort()
        for j, (s, t) in enumerate(taps):
            descs.append((s, bank, t, j == 0, j == len(taps) - 1))
    return mats, descs


def _build(nc, kx, ky):
    """Trace the bass program. kx, ky: 3x3 numpy Sobel kernels."""
    mats, mm_descs = _plan(kx, ky)
    n_mats = len(mats)

    x_d = nc.dram_tensor("x", [B_LOC, C, H, W], F32, kind="ExternalInput")
    w_d = nc.dram_tensor("wts", [HALF, n_mats * HALF], F16, kind="ExternalInput")
    out_d = nc.dram_tensor("out", [B_LOC, C, H, W], F32, kind="ExternalOutput")

    x_flat = x_d[:].rearrange("b c h w -> (b c) h w")
    out_flat = out_d[:].rearrange("b c h w -> (b c) h w")

    out_dmas = []

    with ExitStack() as ctx:
        tc = ctx.enter_context(TileContext(nc))
        wpool = ctx.enter_context(tc.tile_pool(name="wts", bufs=1))
        xpool = ctx.enter_context(tc.tile_pool(name="xin", bufs=10))
        x16pool = ctx.enter_context(tc.tile_pool(name="x16", bufs=6))
        pspool = ctx.enter_context(tc.tile_pool(name="ps", bufs=2, space="PSUM"))
        cpool = ctx.enter_context(tc.tile_pool(name="gxc", bufs=6))
        qpool = ctx.enter_context(tc.tile_pool(name="qg", bufs=5))
        mpool = ctx.enter_context(tc.tile_pool(name="mg", bufs=5))
        opool = ctx.enter_context(tc.tile_pool(name="og", bufs=14))
        spool = ctx.enter_context(tc.tile_pool(name="seam", bufs=1))

        wt = wpool.tile([HALF, n_mats * HALF], F16)
        nc.scalar.dma_start(wt[:], w_d[:])

        # ---- late seam pass: small steps spread across the main loop ----
        sx = spool.tile([128, 4 * WG], F32)   # rows 126..129, guarded
        sxv = sx[:].rearrange("p (r c) -> p r c", r=4)
        seam_steps = []

        def _seam_gather():
            nc.gpsimd.memset(sxv[:, :, 0:WG:WG - 1], 0.0)
            nc.sync.dma_start(
                sxv[:, :, 1:W + 1], x_flat[:, H // 2 - 2:H // 2 + 2, :]
            )

        def vcomb(name, col):
            """v[r] = sum_di col[di] * x[r + di - 1] for output block rows
            1..2 (image rows 127, 128), guarded width. All on GPSIMD
            (tensor_scalar/tensor_tensor only) to keep DVE free."""
            t = spool.tile([128, 2 * WG], F32, tag=f"v_{name}")
            tv = t[:].rearrange("p (r c) -> p r c", r=2)
            rows = [sxv[:, 0:2, :], sxv[:, 1:3, :], sxv[:, 2:4, :]]
            terms = [(float(w), r) for w, r in zip(col, rows) if w != 0.0]
            tmp = spool.tile([128, 2 * WG], F32, tag=f"vt_{name}")
            tmpv = tmp[:].rearrange("p (r c) -> p r c", r=2)

            # dst <- w0*r0; for each extra term: tmp <- w*r, dst <- dst+tmp
            w0, r0 = terms[0]
            seam_steps.append(lambda d=tv, w=w0, r=r0: nc.gpsimd.tensor_scalar(
                d[:], r, w, None, OP.mult))
            for w, r in terms[1:]:
                seam_steps.append(lambda d=tmpv, w=w, r=r:
                                  nc.gpsimd.tensor_scalar(d[:], r, w, None, OP.mult))
                seam_steps.append(lambda d=tv, s=tmpv: nc.gpsimd.tensor_tensor(
                    d[:], d[:], s[:], OP.add))
            return tv

        def hcomb(name, vs):
            """sum_t vs[t] shifted by t over data cols -> [128, 2, W]"""
            ot = spool.tile([128, 2 * W], F32, tag=f"h_{name}")
            otv = ot[:].rearrange("p (r c) -> p r c", r=2)
            items = sorted(vs.items())
            acc = None
            for i, (t, tv) in enumerate(items):
                sh = tv[:, :, 1 + t:1 + t + W]
                if acc is None:
                    if len(items) == 1:
                        seam_steps.append(
                            lambda o=otv, s=sh: nc.gpsimd.tensor_copy(o[:], s))
                    acc = sh
                elif i == len(items) - 1:
                    seam_steps.append(
                        lambda o=otv, a=acc, s=sh:
                        nc.gpsimd.tensor_tensor(o[:], a, s, OP.add))
                else:
                    t2 = spool.tile([128, 2 * W], F32, tag=f"ha_{name}_{i}")
                    t2v = t2[:].rearrange("p (r c) -> p r c", r=2)
                    seam_steps.append(
                        lambda o=t2v, a=acc, s=sh:
                        nc.gpsimd.tensor_tensor(o[:], a, s, OP.add))
                    acc = t2v[:]
            return otv

        kxc = [[float(kx[di, t]) for di in range(3)] for t in range(3)]
        kyc = [[float(ky[di, t]) for di in range(3)] for t in range(3)]
        vgx = {t: vcomb(f"gx{t}", kxc[t + 1]) for t in (-1, 0, 1)
               if any(kxc[t + 1])}
        vgy = {t: vcomb(f"gy{t}", kyc[t + 1]) for t in (-1, 0, 1)
               if any(kyc[t + 1])}
        gxs = hcomb("gx", vgx)
        gys = hcomb("gy", vgy)
        q1s = spool.tile([128, 2 * W], F32)
        q2s = spool.tile([128, 2 * W], F32)
        ms = spool.tile([128, 2 * W], F32)
        os_ = spool.tile([128, 2 * W], F32)
        seam_steps.append(lambda: nc.scalar.activation(
            q1s[:], gxs, AF.Square))
        seam_steps.append(lambda: nc.scalar.activation(
            q2s[:], gys, AF.Square))
        seam_steps.append(lambda: nc.gpsimd.tensor_tensor(
            ms[:], q1s[:], q2s[:], OP.add))
        seam_steps.append(lambda: nc.scalar.activation(
            os_[:], ms[:], AF.Sqrt))
        # rows 127/128 belong exclusively to this scatter (the bulk output
        # DMAs skip them), so it needs no ordering deps and runs mid-stream
        seam_steps.append(lambda: nc.sync.dma_start(
            out_flat[:, H // 2 - 1:H // 2 + 1, :],
            os_[:].rearrange("p (r c) -> p r c", r=2)))

        # ---- main loop over image pairs, software-pipelined emission ----
        # stage A (pair g):   input DMA, convert, matmuls, PSUM evacuations
        # stage B (pair g-1): m = gx^2+gy^2 (DVE), sqrt (ACT)
        # stage C (pair g-1-FLUSH_DELAY): output DMA (SP)
        # This keeps every queue free of waits on results a peer engine is
        # producing in the same pair (the ACT<->DVE ping-pong would otherwise
        # set the pipeline cadence).
        qs, os2 = {}, {}

        def stage_a(g):
            xin = xpool.tile([128, 1024], F32)          # [p][i h w]
            xinv = xin[:].rearrange("p (i h w) -> p i h w", i=2, h=2)
            nc.sync.dma_start(
                xinv[:],
                x_flat[2 * g:2 * g + 2].rearrange("i (h p) w -> p i h w", p=128),
            )
            x16 = x16pool.tile([128, 4 * WG], F16)      # [p][i h c], guarded
            x16v = x16[:].rearrange("p (i h c) -> p i h c", i=2, h=2)
            # zero the guard columns, then convert fp32 -> fp16 on GPSIMD
            nc.gpsimd.memset(x16v[:, :, :, 0:WG:WG - 1], 0.0)
            nc.gpsimd.tensor_copy(x16v[:, :, :, 1:W + 1], xinv[:])

            q = qpool.tile([128, 2048], F16)            # [p][gx A,B | gy A,B]
            g16 = cpool.tile([128, 1024], F16)
            # PSUM pair tile: A_gy | B_gy | A_gx | B_gx (gy first so the ACT
            # square starts before the pair's burst finishes; all evac ops
            # read/write contiguous 1024-wide spans)
            ps = pspool.tile([128, 2048], F32)
            for bank, _kern in ((0, None), (1, None)):
                for i in range(2):
                    for slot, bk, t, start, stop in mm_descs:
                        if bk != bank:
                            continue
                        nc.tensor.matmul(
                            ps[:, bank * 1024 + i * 512:bank * 1024 + (i + 1) * 512],
                            wt[:, slot * HALF:(slot + 1) * HALF],
                            x16v[:, i, :, t + 1:t + 1 + W],
                            start=start,
                            stop=stop,
                            skip_group_check=True,
                        )
            # gy both images: Square straight out of PSUM (ACT, f16 out)
            nc.scalar.activation(q[:, 1024:2048], ps[:, 0:1024], AF.Square)
            # gx both images: PSUM -> f16 copy, square on DVE
            nc.vector.tensor_copy(g16[:], ps[:, 1024:2048])
            nc.vector.tensor_tensor(q[:, 0:1024], g16[:], g16[:], OP.mult)
            qs[g] = q

        def stage_b(g):
            q = qs.pop(g)
            m = mpool.tile([128, 1024], F16)            # [p][i c]
            nc.vector.tensor_tensor(
                m[:], q[:, 0:1024], q[:, 1024:2048], OP.add)
            o = opool.tile([128, 1024], F32)
            nc.scalar.activation(o[:], m[:], AF.Sqrt)
            os2[g] = o

        def stage_c(g):
            # Output DMAs on SP, FLUSH_DELAY pairs after the sqrt: by issue
            # time the data is long ready, so they never head-of-line block
            # the input stream sharing SP. Every pair is written as two DMAs
            # that SKIP rows 127/128 -- the seam scatter alone owns those
            # rows, so it needs no ordering deps and the duplicate write is
            # gone from the HBM traffic.
            o = os2.pop(g)
            ov = o[:].rearrange("p (i h w) -> p i h w", i=2, h=2)
            nc.sync.dma_start(
                out_flat[2 * g:2 * g + 2, 0:HALF - 1, :].rearrange(
                    "i p w -> p i w"),
                ov[0:HALF - 1, :, 0, :],
            )
            nc.sync.dma_start(
                out_flat[2 * g:2 * g + 2, HALF + 1:H, :].rearrange(
                    "i p w -> p i w"),
                ov[1:HALF, :, 1, :],
            )

        for g in range(PAIRS + 1 + FLUSH_DELAY):
            # stage B first: m(g-1)/sqrt(g-1) are ready to run, so they sit
            # ahead of pair g's PSUM evacuations in the DVE/ACT queues
            # without delaying them (the evacuations wait on pair g's
            # matmuls anyway).
            if 0 <= g - 1 < PAIRS:
                stage_b(g - 1)
            if g < PAIRS:
                stage_a(g)
            if 0 <= g - 1 - FLUSH_DELAY < PAIRS:
                stage_c(g - 1 - FLUSH_DELAY)
            if g == 0:
                # right behind in(0) on SP: lands early, so the seam compute
                # steps below never head-of-line-block the Pool queue
                _seam_gather()
            if g >= 4 and seam_steps:
                # wait-until floor stops the scheduler from hoisting seam
                # work ahead of the warmup-critical converts
                with tc.tile_wait_until(0.003 * g):
                    seam_steps.pop(0)()
        while seam_steps:
            seam_steps.pop(0)()
    return nc


def _make_weights(kx, ky):
    mats, _descs = _plan(kx, ky)
    w = np.zeros((HALF, len(mats) * HALF), dtype=np.float16)
    for i, V in enumerate(mats):
        w[:, i * HALF:(i + 1) * HALF] = V.astype(np.float16)
    return w


def kernel(x, sobel_x, sobel_y):
    x = np.asarray(x)
    kx = np.asarray(sobel_x).reshape(3, 3).astype(np.float32)
    ky = np.asarray(sobel_y).reshape(3, 3).astype(np.float32)

    nc = bacc.Bacc()
    _build(nc, kx, ky)
    nc.compile()

    wts = _make_weights(kx, ky)
    in_maps = [
        {"x": np.ascontiguousarray(x[i * B_LOC:(i + 1) * B_LOC]), "wts": wts}
        for i in range(N_CORES)
    ]
    res = run_bass_kernel_spmd(nc, in_maps, core_ids=list(range(N_CORES)))
    global LAST_RESULTS
    LAST_RESULTS = res
    return np.concatenate([r["out"] for r in res.results], axis=0)


LAST_RESULTS = None
